# revision 1
# baseline (speedup 1.0000x reference)
"""GAT layer (PyG GATConv H=4,C=64 + PReLU) on 8 Trainium2 NeuronCores.

Strategy (graph/data parallel, dst-sharded):
  - Host: add self loops, sort edges by dst, partition dst-nodes across 8
    cores (6250 each), group each core's edges into 128-dst "blocks", tile
    each block's edges into 128-edge tiles.  Tile/chunk structure is made
    uniform across cores so ONE SPMD program serves all 8 cores; all
    per-core divergence rides in input data.
  - Node rotation: core m's table stores node (m*6250 + l) mod N at row
    l+1, fed by a host-rotated copy of x.  Hence every core's OWN dst
    nodes are rows 1..6250 — the per-edge a_dst gather uses one small
    int16-indexed window identical on all cores.
  - Phase 1 (per core, replicated matmul): table row = [a_dst(4) | h(256)
    | a_src(4) | junk] (bf16, 768B stride).  h = x @ lin_w.T on PE;
    a_src/a_dst fold into the same matmul as extra columns (w_a =
    lin_w.T @ att).  Rows 0 and N+1 are sentinels with a_src=-30000 so
    padded edges get p = exp(lrelu(-big)) = 0.
  - Phase 2: per 128-edge tile, dma_gather table rows by src (two int16
    windows, lo/hi), one-hot S1[k,slot] = (rel_dst_k == slot) built by
    iota-compare, p = exp(leaky_relu(a_src+a_dst)), messages h*p
    scatter-added into the block's 128 dst slots by matmul
    (lhsT=S1, rhs=[h*p | p]), accumulating [dst,256 msg | 4 denom] in
    PSUM across the block's tiles.  Epilogue: out = prelu(msg/denom+bias).
  - Softmax max-subtraction is skipped (logits are O(1); exp can't
    overflow) making the edge pass single-sweep: out = (Σ p·h)/(Σ p).
"""

import sys

sys.path.insert(0, "/opt/trn_rl_repo")

import numpy as np
import ml_dtypes

import concourse.bass as bass
import concourse.bacc as bacc
import concourse.tile as tile
from concourse import mybir
from concourse.bass import AP

F32 = mybir.dt.float32
BF16 = mybir.dt.bfloat16
I16 = mybir.dt.int16
AF = mybir.ActivationFunctionType
OP = mybir.AluOpType
BF16NP = ml_dtypes.bfloat16

P = 128
NEG_SLOPE = 0.2
SENT_NEG = -30000.0


class Cfg:
    def __init__(self, n_nodes=50000, in_ch=512, hid=64, heads=4, n_cores=8,
                 tc_max=8, node_chunk=4096, win=32768, adst_segs=8):
        assert n_nodes % n_cores == 0
        assert in_ch % P == 0
        self.n_nodes = n_nodes
        self.in_ch = in_ch
        self.hid = hid
        self.heads = heads
        self.hc = hid * heads                      # 256
        self.rowp = self.hc + 2 * heads            # 264 payload cols
        self.row = 384                             # table row stride (768B)
        self.gcol = self.row                       # gather full 768B rows
        self.n_cores = n_cores
        self.ndst = n_nodes // n_cores             # 6250
        self.nblk = -(-self.ndst // P)             # 49
        self.kt = in_ch // P                       # 4
        self.tc_max = tc_max
        self.node_chunk = node_chunk
        self.rows = n_nodes + 2                    # + two sentinel rows
        self.win = min(win, self.rows)             # int16 gather window
        assert self.rows <= 2 * self.win, "lo+hi windows must cover table"
        self.adst_segs = adst_segs


CFG = Cfg()


def _wrap16(flat):
    """int16 index list -> dma_gather layout [128, n/16] (i -> [i%16, i//16],
    replicated to all 8 Q7 core groups)."""
    n = len(flat)
    assert n % 16 == 0
    a = np.asarray(flat, dtype=np.int16).reshape(n // 16, 16).T  # [16, n/16]
    return np.tile(a, (8, 1))                                    # [128, n/16]


# ---------------------------------------------------------------- host prep

def host_prep_edges(edge_index, cfg):
    """Sort/partition/tile edges.  Returns (per_core data dicts, meta)."""
    n = cfg.n_nodes
    loop = np.arange(n, dtype=np.int64)
    src = np.concatenate([edge_index[0].astype(np.int64), loop])
    dst = np.concatenate([edge_index[1].astype(np.int64), loop])
    order = np.argsort(dst, kind="stable")
    src_s = src[order]
    dst_s = dst[order]

    lo_rows = cfg.win                 # lo window = rows [0, win)
    hi_base = cfg.rows - cfg.win      # hi window = rows [hi_base, rows)
    sent_hi_rel = cfg.rows - 1 - hi_base

    # per-(core, block) segments; rotated src rows; lo/hi split
    seg = {}
    tl_req = np.zeros((cfg.n_cores, cfg.nblk), dtype=np.int64)
    th_req = np.zeros((cfg.n_cores, cfg.nblk), dtype=np.int64)
    for m in range(cfg.n_cores):
        base = m * cfg.ndst
        for b in range(cfg.nblk):
            d0 = base + b * P
            d1 = min(base + (b + 1) * P, base + cfg.ndst)
            lo = np.searchsorted(dst_s, d0)
            hi = np.searchsorted(dst_s, d1)
            s_rot = (src_s[lo:hi] - base) % n + 1     # rotated table row
            d_loc = dst_s[lo:hi] - d0                 # slot in block
            a_idx = dst_s[lo:hi] - base + 1           # rotated a_dst row
            is_lo = s_rot < lo_rows
            seg[(m, b)] = (s_rot, d_loc, a_idx, is_lo)
            tl_req[m, b] = -(-int(is_lo.sum()) // P)
            th_req[m, b] = -(-int((~is_lo).sum()) // P)
    tl = tl_req.max(axis=0).astype(int)
    th = th_req.max(axis=0).astype(int)
    for b in range(cfg.nblk):
        if tl[b] + th[b] == 0:
            tl[b] = 1

    # uniform chunk structure: per block, lo tiles then hi tiles,
    # split at tc_max
    chunks = []          # (block, t0_global, ntiles, nidx, icol0, window)
    tiles_b = tl + th
    col0 = np.concatenate([[0], np.cumsum(tiles_b)])[:-1].astype(int)
    T = int(tiles_b.sum())
    icol = 0
    for b in range(cfg.nblk):
        t0 = int(col0[b])
        for half, nt_half in (("lo", int(tl[b])), ("hi", int(th[b]))):
            q0 = 0
            while q0 < nt_half:
                qq = min(cfg.tc_max, nt_half - q0)
                chunks.append(dict(b=b, t0=t0 + q0, nt=qq, nidx=qq * P,
                                   icol=icol, half=half))
                icol += qq * P // 16
                q0 += qq
            t0 += nt_half
    icol_main = icol

    # a_dst gather segments: ~equal tile ranges, capped at 8 tiles (1024
    # descriptors = the SWDGE ring) per dma_gather instruction
    nseg = max(min(cfg.adst_segs, T), -(-T // 8))
    seg_bounds = [round(i * T / nseg) for i in range(nseg + 1)]
    adst_segs = []
    acol = 0
    for i in range(nseg):
        ts, te = seg_bounds[i], seg_bounds[i + 1]
        if te <= ts:
            continue
        adst_segs.append(dict(ts=ts, nt=te - ts, nidx=(te - ts) * P,
                              icol=acol))
        acol += (te - ts) * P // 16
    icol_adst = acol

    per_core = []
    for m in range(cfg.n_cores):
        rel_all = np.zeros((P, T), dtype=np.float32)
        aidx_all = np.zeros((P, T), dtype=np.int64)
        midx_all = np.zeros((P, icol_main), dtype=np.int16)
        # fill per block
        tbuf = {}
        for b in range(cfg.nblk):
            s_rot, d_loc, a_idx, is_lo = seg[(m, b)]
            for half, nt_half in (("lo", int(tl[b])), ("hi", int(th[b]))):
                sel = is_lo if half == "lo" else ~is_lo
                ne = int(sel.sum())
                npad = nt_half * P
                if half == "lo":
                    bs = np.zeros(npad, dtype=np.int64)       # sentinel_lo
                else:
                    bs = np.full(npad, hi_base + sent_hi_rel, dtype=np.int64)
                br = np.zeros(npad, dtype=np.int64)
                ba = np.zeros(npad, dtype=np.int64)
                bs[:ne] = s_rot[sel]
                br[:ne] = d_loc[sel]
                ba[:ne] = a_idx[sel]
                tbuf[(b, half)] = (bs, br, ba)
        # place tiles into global arrays
        for b in range(cfg.nblk):
            t0 = int(col0[b])
            for half, nt_half in (("lo", int(tl[b])), ("hi", int(th[b]))):
                if nt_half == 0:
                    continue
                bs, br, ba = tbuf[(b, half)]
                rel_all[:, t0:t0 + nt_half] = br.reshape(nt_half, P).T
                aidx_all[:, t0:t0 + nt_half] = ba.reshape(nt_half, P).T
                t0 += nt_half
        for ch in chunks:
            b = ch["b"]
            half = ch["half"]
            bs, _, _ = tbuf[(b, half)]
            # local tile offset within this half
            base_t = int(col0[b]) + (int(tl[b]) if half == "hi" else 0)
            off = (ch["t0"] - base_t) * P
            flat = bs[off:off + ch["nidx"]].copy()
            if half == "hi":
                flat -= hi_base
            assert flat.min() >= 0 and flat.max() < cfg.win, (
                flat.min(), flat.max(), half)
            midx_all[:, ch["icol"]:ch["icol"] + ch["nidx"] // 16] = \
                _wrap16(flat)
        aidx16 = np.zeros((P, icol_adst), dtype=np.int16)
        for sg in adst_segs:
            flat = aidx_all[:, sg["ts"]:sg["ts"] + sg["nt"]].T.ravel()
            aidx16[:, sg["icol"]:sg["icol"] + sg["nidx"] // 16] = \
                _wrap16(flat)
        rel2 = np.repeat(rel_all.astype(BF16NP), 2, axis=1)   # [P, 2T]
        per_core.append(dict(
            midx=np.ascontiguousarray(midx_all),
            aidx=np.ascontiguousarray(aidx16),
            rel=np.ascontiguousarray(rel2),
        ))
    meta = dict(chunks=chunks, tiles_b=tiles_b, col0=col0, T=T,
                icol_main=icol_main, icol_adst=icol_adst,
                adst_segs=adst_segs, hi_base=hi_base)
    return per_core, meta


def host_prep_weights(x, lin_w, att_src, att_dst, bias, prelu_w, cfg):
    n, ic, h, c = cfg.n_nodes, cfg.in_ch, cfg.heads, cfg.hid
    w3 = lin_w.astype(np.float64).reshape(h, c, ic)
    wa_src = (w3 * att_src.astype(np.float64).reshape(h, c, 1)).sum(1).T
    wa_dst = (w3 * att_dst.astype(np.float64).reshape(h, c, 1)).sum(1).T
    lwT = lin_w.astype(np.float64).T                           # [ic, 256]
    lwTi = lwT.reshape(ic, h, c).transpose(0, 2, 1).reshape(ic, h * c)
    rhs = np.concatenate([wa_dst, lwTi, wa_src], axis=1)       # [ic, 264]
    rhs_w = np.ascontiguousarray(rhs.astype(BF16NP))
    def inter(v):
        return v.reshape(h, c).T.reshape(h * c)
    bias_rep = np.ascontiguousarray(np.broadcast_to(
        inter(bias.astype(np.float32)), (P, cfg.hc)))
    pw_rep = np.ascontiguousarray(np.broadcast_to(
        inter(prelu_w.astype(np.float32)), (P, cfg.hc)))
    sent = np.zeros((2, cfg.rowp), dtype=BF16NP)
    sent[:, cfg.rowp - cfg.heads:] = SENT_NEG      # a_src cols
    xbf = x.astype(BF16NP)
    xTs = []
    for m in range(cfg.n_cores):
        r = np.roll(xbf, -m * cfg.ndst, axis=0)
        xTs.append(np.ascontiguousarray(r.T))
    return dict(rhs_w=rhs_w, bias_rep=bias_rep, pw_rep=pw_rep, sent=sent,
                xTs=xTs)


# ---------------------------------------------------------------- builder

def build(cfg, meta, bias_nonzero=True, parts=None):
    parts = parts or {"p1", "gather", "adst", "s1", "pcomp", "mm", "epi"}
    n, row, hc, h = cfg.n_nodes, cfg.row, cfg.hc, cfg.heads
    nc = bacc.Bacc()

    xT = nc.declare_dram_parameter("xT", [cfg.in_ch, n], BF16, isOutput=False)
    rhs_w = nc.declare_dram_parameter("rhs_w", [cfg.in_ch, cfg.rowp], BF16,
                                      isOutput=False)
    bias_rep = nc.declare_dram_parameter("bias_rep", [P, hc], F32,
                                         isOutput=False)
    pw_rep = nc.declare_dram_parameter("pw_rep", [P, hc], F32, isOutput=False)
    sent = nc.declare_dram_parameter("sent", [2, cfg.rowp], BF16,
                                     isOutput=False)
    midx = nc.declare_dram_parameter("midx", [P, meta["icol_main"]], I16,
                                     isOutput=False)
    aidx = nc.declare_dram_parameter("aidx", [P, meta["icol_adst"]], I16,
                                     isOutput=False)
    reldst = nc.declare_dram_parameter("reldst", [P, 2 * meta["T"]], BF16,
                                       isOutput=False)
    out = nc.declare_dram_parameter("out", [cfg.ndst, hc], F32, isOutput=True)

    table = nc.dram_tensor("table", [cfg.rows, row], BF16)
    T = meta["T"]
    hi_base = meta["hi_base"]

    with tile.TileContext(nc) as tc:
        fpool_cm = tc.tile_pool(name="p2f", bufs=1)
        apool_cm = tc.tile_pool(name="p2a", bufs=2)
        fpool = fpool_cm.__enter__()
        apool = apool_cm.__enter__()

        midx_sb = fpool.tile([P, meta["icol_main"]], I16)
        nc.sync.dma_start(out=midx_sb[:], in_=midx[:, :])
        aidx_sb = fpool.tile([P, meta["icol_adst"]], I16)
        nc.sync.dma_start(out=aidx_sb[:], in_=aidx[:, :])
        rel_sb = fpool.tile([P, 2 * T], BF16)
        nc.sync.dma_start(out=rel_sb[:], in_=reldst[:, :])
        bias_sb = fpool.tile([P, hc], F32)
        nc.sync.dma_start(out=bias_sb[:], in_=bias_rep[:, :])
        pw_sb = fpool.tile([P, hc], F32)
        nc.sync.dma_start(out=pw_sb[:], in_=pw_rep[:, :])

        iota_i = fpool.tile([P, P], mybir.dt.int32)
        nc.gpsimd.iota(iota_i[:], pattern=[[1, P]], base=0,
                       channel_multiplier=0)
        iota_f = fpool.tile([P, P], F32)
        nc.vector.tensor_copy(out=iota_f[:], in_=iota_i[:])
        iota_bf = fpool.tile([P, P], BF16)
        nc.vector.tensor_copy(out=iota_bf[:], in_=iota_f[:])
        adst_edge = fpool.tile([P, T, h], BF16)

        def emit_adst_gathers():
            arows = cfg.ndst + 2
            for sg in (meta["adst_segs"] if "adst" in parts else []):
                aseg = apool.tile([P, sg["nt"], P], BF16, tag="aseg")
                nc.gpsimd.dma_gather(
                    out_ap=aseg[:],
                    in_ap=table[0:arows, 0:P],
                    idxs_ap=aidx_sb[:, sg["icol"]:sg["icol"]
                                    + sg["nidx"] // 16],
                    num_idxs=sg["nidx"],
                    num_idxs_reg=sg["nidx"],
                    elem_size=P,
                    elem_step=row)
                nc.vector.tensor_copy(
                    out=adst_edge[:, sg["ts"]:sg["ts"] + sg["nt"], :],
                    in_=aseg[:, :, 0:h])

        # ---------------- phase 1: build table ----------------
        with (
            tc.tile_pool(name="p1w", bufs=1) as wpool,
            tc.tile_pool(name="p1x", bufs=2) as xpool,
            tc.tile_pool(name="p1o", bufs=4) as opool,
            tc.tile_pool(name="p1ps", bufs=4, space="PSUM") as pspool,
        ):
            w_sb = wpool.tile([P, cfg.kt, cfg.rowp], BF16)
            nc.sync.dma_start(
                out=w_sb[:],
                in_=rhs_w[:, :].rearrange("(k p) r -> p k r", p=P))
            nc.sync.dma_start(out=table[0:1, 0:cfg.rowp], in_=sent[0:1, :])
            nc.sync.dma_start(out=table[cfg.rows - 1:cfg.rows, 0:cfg.rowp],
                              in_=sent[1:2, :])

            nch = cfg.node_chunk
            p1_starts = list(range(0, n if "p1" in parts else 0, nch))

            def p1_chunk(n0):
                nn = min(nch, n - n0)
                xg = xpool.tile([P, cfg.kt, nch], BF16, tag="xg")
                nc.sync.dma_start(
                    out=xg[:, :, :nn],
                    in_=xT[:, n0:n0 + nn].rearrange("(k p) q -> p k q", p=P))
                for t0 in range(0, nn, P):
                    mm = min(P, nn - t0)
                    ps = pspool.tile([P, cfg.rowp], F32, tag="ps")
                    for k in range(cfg.kt):
                        nc.tensor.matmul(
                            ps[:mm, :],
                            lhsT=xg[:, k, t0:t0 + mm],
                            rhs=w_sb[:, k, :],
                            start=(k == 0), stop=(k == cfg.kt - 1))
                    hrow = opool.tile([P, cfg.rowp], BF16, tag="hrow")
                    nc.scalar.copy(out=hrow[:mm, :], in_=ps[:mm, :])
                    nc.sync.dma_start(
                        out=table[1 + n0 + t0:1 + n0 + t0 + mm, 0:cfg.rowp],
                        in_=hrow[:mm, :])

            # chunk 0 covers this core's own dst rows (ndst+2 <= node_chunk
            # + 1): emit it, barrier, then emit the a_dst gathers so they
            # overlap the remaining phase-1 chunks.
            early_adst = bool(p1_starts) and cfg.ndst + 2 <= nch + 1
            if p1_starts:
                p1_chunk(p1_starts[0])
            if early_adst:
                tc.strict_bb_all_engine_barrier()
                emit_adst_gathers()
            for n0 in p1_starts[1:]:
                p1_chunk(n0)

        # barrier: all table rows written before the main gathers read them
        tc.strict_bb_all_engine_barrier()

        if not early_adst:
            emit_adst_gathers()

        # ---------------- phase 2: edge pass ----------------
        with (
            tc.tile_pool(name="p2g", bufs=3) as gpool,
            tc.tile_pool(name="p2s", bufs=3) as s1pool,
            tc.tile_pool(name="p2e", bufs=3) as epool,
            tc.tile_pool(name="p2o", bufs=3) as obpool,
            tc.tile_pool(name="p2ps", bufs=2, space="PSUM") as ps2pool,
        ):
            # block loop
            blk_chunks = {}
            for chi, ch in enumerate(meta["chunks"]):
                ch["qn"] = chi % 4
                blk_chunks.setdefault(ch["b"], []).append(ch)
            for b in range(cfg.nblk):
                chs = blk_chunks[b]
                ps = ps2pool.tile([P, hc + h], F32, tag="psb")
                for ci, ch in enumerate(chs):
                    qq = ch["nt"]
                    t0 = ch["t0"]
                    g = gpool.tile([P, qq, cfg.gcol], BF16, tag="g")
                    if "gather" in parts:
                        if ch["half"] == "lo":
                            in_ap = table[0:cfg.win, :]
                        else:
                            in_ap = table[hi_base:cfg.rows, :]
                        nc.gpsimd.dma_gather(
                            out_ap=g[:],
                            in_ap=in_ap,
                            idxs_ap=midx_sb[:, ch["icol"]:ch["icol"]
                                            + ch["nidx"] // 16],
                            num_idxs=ch["nidx"],
                            num_idxs_reg=ch["nidx"],
                            elem_size=cfg.gcol,
                            elem_step=row)

                    # one-hot S1[k, q, slot] = (rel[k, q] == slot)
                    s1 = s1pool.tile([P, qq, P], BF16, tag="s1")
                    if "s1" in parts:
                        rsl = rel_sb[:, 2 * t0:2 * (t0 + qq)]
                        rel_b = AP(rsl.tensor, rsl.offset,
                                   [rsl.ap[0], [2, qq], [0, P // 2], [1, 2]])
                        iap = iota_bf[:]
                        iota_b = AP(iap.tensor, iap.offset,
                                    [iap.ap[0], [0, qq], [2, P // 2], [1, 2]])
                        s1v = s1[:]
                        s1_b = AP(s1v.tensor, s1v.offset,
                                  [s1v.ap[0], [P, qq], [2, P // 2], [1, 2]])
                        nc.vector.tensor_tensor(
                            out=s1_b, in0=rel_b, in1=iota_b, op=OP.is_equal)

                    if "pcomp" in parts:
                        # p = exp(leaky_relu(a_src + a_dst))
                        ef = epool.tile([P, qq, h], F32, tag="ef")
                        nc.vector.tensor_add(
                            out=ef[:],
                            in0=g[:, :, 4 + hc:4 + hc + h],
                            in1=adst_edge[:, t0:t0 + qq, :])
                        ng = epool.tile([P, qq, h], F32, tag="ng")
                        nc.vector.tensor_scalar(
                            out=ng[:], in0=ef[:], scalar1=0.0,
                            scalar2=NEG_SLOPE, op0=OP.min, op1=OP.mult)
                        nc.vector.scalar_tensor_tensor(
                            out=ef[:], in0=ef[:], scalar=0.0,
                            op0=OP.max, in1=ng[:], op1=OP.add)
                        nc.scalar.activation(out=ef[:], in_=ef[:],
                                             func=AF.Exp)
                        nc.vector.tensor_copy(
                            out=g[:, :, 4 + hc:4 + hc + h], in_=ef[:])
                        # msg = h * p; h interleaved [c, hh] so the
                        # broadcast AP keeps a stride-1 last dim (2x DVE)
                        msg = g[:, :, 4:4 + hc].rearrange(
                            "p q (c hh) -> p q c hh", hh=h)
                        pslice = g[:, :, 4 + hc:4 + hc + h]
                        pb = AP(pslice.tensor, pslice.offset,
                                [pslice.ap[0], pslice.ap[1],
                                 [0, cfg.hid], [1, h]])
                        nc.vector.tensor_tensor(out=msg, in0=msg, in1=pb,
                                                op=OP.mult)

                    if "mm" in parts:
                        for j in range(qq):
                            nc.tensor.matmul(
                                ps[:, :],
                                lhsT=s1[:, j, :],
                                rhs=g[:, j, 4:4 + hc + h],
                                start=(ci == 0 and j == 0),
                                stop=(ci == len(chs) - 1 and j == qq - 1))

                # epilogue: out = prelu(msg/denom + bias)
                if "epi" not in parts or "mm" not in parts:
                    continue
                den = epool.tile([P, h], F32, tag="den")
                nc.vector.tensor_scalar_add(out=den[:], in0=ps[:, hc:hc + h],
                                            scalar1=1e-6)
                rec = epool.tile([P, h], F32, tag="rec")
                nc.vector.reciprocal(out=rec[:], in_=den[:])
                ob = obpool.tile([P, hc], F32, tag="ob")
                recb = AP(rec.tensor, rec[:].offset,
                          [rec[:].ap[0], [0, cfg.hid], [1, h]])
                nc.vector.tensor_tensor(
                    out=ob[:].rearrange("p (c hh) -> p c hh", hh=h),
                    in0=ps[:, 0:hc].rearrange("p (c hh) -> p c hh", hh=h),
                    in1=recb, op=OP.mult)
                if bias_nonzero:
                    nc.vector.tensor_add(out=ob[:], in0=ob[:], in1=bias_sb[:])
                t2 = obpool.tile([P, hc], F32, tag="t2")
                nc.vector.scalar_tensor_tensor(
                    out=t2[:], in0=ob[:], scalar=0.0, op0=OP.min,
                    in1=pw_sb[:], op1=OP.mult)
                obp = obpool.tile([P, hc], F32, tag="obp")
                obpv = obp[:]
                # write through a permuted view: col c*4+hh -> hh*64+c
                obp_perm = AP(obpv.tensor, obpv.offset,
                              [obpv.ap[0], [cfg.hid, h], [1, cfg.hid]])
                iview = [[1, h], [h, cfg.hid]]
                ob_i = AP(ob[:].tensor, ob[:].offset, [ob[:].ap[0]] + iview)
                t2_i = AP(t2[:].tensor, t2[:].offset, [t2[:].ap[0]] + iview)
                nc.vector.scalar_tensor_tensor(
                    out=obp_perm, in0=ob_i, scalar=0.0, op0=OP.max,
                    in1=t2_i, op1=OP.add)
                rows = min(P, cfg.ndst - b * P)
                nc.sync.dma_start(out=out[b * P:b * P + rows, :],
                                  in_=obp[:rows, :])
        apool_cm.__exit__(None, None, None)
        fpool_cm.__exit__(None, None, None)
    return nc


# ---------------------------------------------------------------- runner

def _prepare(x, edge_index, lin_w, att_src, att_dst, bias, prelu_w, cfg):
    per_core, meta = host_prep_edges(np.asarray(edge_index), cfg)
    shared = host_prep_weights(np.asarray(x), np.asarray(lin_w),
                               np.asarray(att_src), np.asarray(att_dst),
                               np.asarray(bias), np.asarray(prelu_w), cfg)
    bias_nonzero = bool(np.any(np.asarray(bias) != 0))
    nc = build(cfg, meta, bias_nonzero=bias_nonzero)
    in_maps = []
    for m in range(cfg.n_cores):
        im = dict(rhs_w=shared["rhs_w"], bias_rep=shared["bias_rep"],
                  pw_rep=shared["pw_rep"], sent=shared["sent"],
                  xT=shared["xTs"][m])
        im["midx"] = per_core[m]["midx"]
        im["aidx"] = per_core[m]["aidx"]
        im["reldst"] = per_core[m]["rel"]
        in_maps.append(im)
    return nc, in_maps


def _run_pjrt(nc, in_maps, n_cores, time_iters=0):
    """Mirror of bass2jax.run_bass_via_pjrt that keeps the compiled callable
    so warm re-executions can be timed (no NTFF profiling in this container).
    Returns (per-core result dicts, min warm wall ns or None)."""
    import time
    import jax
    from jax.sharding import Mesh, PartitionSpec
    from jax.experimental.shard_map import shard_map
    from concourse import bass2jax, mybir as mb

    bass2jax.install_neuronx_cc_hook()
    assert nc.dbg_addr is None
    partition_name = (nc.partition_id_tensor.name
                      if nc.partition_id_tensor else None)
    in_names, out_names, out_avals, zero_outs = [], [], [], []
    for alloc in nc.m.functions[0].allocations:
        if not isinstance(alloc, mb.MemoryLocationSet):
            continue
        name = alloc.memorylocations[0].name
        if alloc.kind == "ExternalInput":
            if name != partition_name:
                in_names.append(name)
        elif alloc.kind == "ExternalOutput":
            shape = tuple(alloc.tensor_shape)
            dtype = mb.dt.np(alloc.dtype)
            out_names.append(name)
            out_avals.append(jax.core.ShapedArray(shape, dtype))
            zero_outs.append(np.zeros(shape, dtype))
    n_params = len(in_names)
    in_names.extend(out_names)
    if partition_name is not None:
        in_names.append(partition_name)

    def _body(*args):
        operands = list(args)
        if partition_name is not None:
            operands.append(bass2jax.partition_id_tensor())
        outs = bass2jax._bass_exec_p.bind(
            *operands,
            out_avals=tuple(out_avals),
            in_names=tuple(in_names),
            out_names=tuple(out_names),
            lowering_input_output_aliases=(),
            sim_require_finite=True,
            sim_require_nnan=True,
            nc=nc,
        )
        return tuple(outs)

    devices = jax.devices()[:n_cores]
    mesh = Mesh(np.asarray(devices), ("core",))
    n_outs = len(out_avals)
    in_specs = (PartitionSpec("core"),) * (n_params + n_outs)
    out_specs = (PartitionSpec("core"),) * n_outs
    sharded = jax.jit(
        shard_map(_body, mesh=mesh, in_specs=in_specs, out_specs=out_specs,
                  check_rep=False),
        keep_unused=True,
    )
    per_core = [[np.asarray(m[name]) for name in in_names[:n_params]]
                for m in in_maps]
    concat_in = [
        np.concatenate([per_core[c][i] for c in range(n_cores)], axis=0)
        for i in range(n_params)
    ]
    concat_zeros = [
        np.zeros((n_cores * z.shape[0], *z.shape[1:]), z.dtype)
        for z in zero_outs
    ]
    sh = jax.sharding.NamedSharding(mesh, PartitionSpec("core"))
    dev_args = [jax.device_put(a, sh) for a in concat_in + concat_zeros]
    out_arrs = sharded(*dev_args)
    jax.block_until_ready(out_arrs)
    t_ns = None
    if time_iters > 0:
        # The axon RPC floor (~90 ms) swamps single-dispatch wall time, but
        # async dispatches pipeline on the device queue: time M back-to-back
        # executions blocking once, for two M values, and use the slope.
        def loop_wall(mreps):
            best = None
            for _ in range(time_iters):
                t0 = time.perf_counter_ns()
                o = None
                for _ in range(mreps):
                    o = sharded(*dev_args)
                jax.block_until_ready(o)
                dt = time.perf_counter_ns() - t0
                best = dt if best is None else min(best, dt)
            return best

        m1, m2 = 4, 20
        w1 = loop_wall(m1)
        w2 = loop_wall(m2)
        t_ns = max(0, (w2 - w1) // (m2 - m1))
    results = [
        {name: np.asarray(out_arrs[i]).reshape(n_cores, *out_avals[i].shape)[c]
         for i, name in enumerate(out_names)}
        for c in range(n_cores)
    ]
    return results, t_ns


def run(x, edge_index, lin_w, att_src, att_dst, bias, prelu_w,
        cfg=None, time_iters=0):
    cfg = cfg or CFG
    nc, in_maps = _prepare(x, edge_index, lin_w, att_src, att_dst, bias,
                           prelu_w, cfg)
    nc.finalize()
    results, t_ns = _run_pjrt(nc, in_maps, cfg.n_cores,
                              time_iters=time_iters)
    outs = [results[m]["out"] for m in range(cfg.n_cores)]
    full = np.concatenate(outs, axis=0).astype(np.float32)
    return full, t_ns


def kernel(**inputs):
    full, _ = run(inputs["x"], inputs["edge_index"], inputs["lin_w"],
                  inputs["att_src"], inputs["att_dst"], inputs["bias"],
                  inputs["prelu_w"])
    return full



# revision 10
# speedup vs baseline: 2.4584x; 2.4584x over previous
"""GAT layer (PyG GATConv H=4,C=64 + PReLU) on 8 Trainium2 NeuronCores, v2.

Strategy (graph/data parallel, dst-sharded; SPMD: one program, per-core data):
  - Host: add self loops, sort edges by dst, partition dst-nodes across 8
    cores (6250 each), group each core's edges into 128-dst blocks, tile into
    128-edge tiles, chunk tiles (<=8) for gathers.  Tile counts maxed across
    cores so ONE program serves all 8; divergence rides in input data.
  - Node rotation: core m handles dst nodes [m*6250, (m+1)*6250); inputs are
    host-rotated so its own dsts are local nodes 0..6249.
  - Table: h rows only (256 bf16 = 512B, gather-size-aligned), stored in
    permuted layout addr(node) = ((node%128)*417 + node//128)*512B so the
    phase-1 writes are contiguous 16KB-per-partition runs (128 descriptors
    per 4096-node chunk instead of 4096).
  - Phase 1: h = x @ lin_w.T on PE (nodes padded to 13*4096); rhs also
    carries folded a_dst columns (lin_w.T @ att_dst); a_dst of the core's
    own 6250 dsts is captured straight from PSUM into SBUF (adst_sb).
  - Phase 2 per 128-edge tile: dma_gather h rows by src (one int16-indexed
    window pair lo/hi, 512B elems, single_packet=False, striped over 4 SWDGE
    queues -- gathers are Q7 descriptor-generation bound at ~6-8ns/desc, so
    descriptor count, not bytes, is the budget).
      a_src = sum_c h*att_src on DVE (recomputed from gathered h; sentinel
        rows hold h with sum h*att_src = -3000 so padded edges get p = 0).
      a_dst per edge descriptor-free: replicate rel_dst across partitions
        via a K=1 PE matmul (ones x rel row), build the slot-major one-hot
        S1T = (iota_p == rel) on DVE, then ef_psum = S1T^T @ adst_blk on PE.
      p = exp(leaky_relu(a_src + a_dst)); messages h*p scatter-added into
        the block's 128 dst slots by matmul (lhsT=S1 edge-major one-hot,
        rhs=[h*p | p]) accumulating [256 msg | 4 denom] in PSUM.
  - Softmax max-subtraction is skipped (logits are O(1)); out =
    prelu((sum p*h)/(sum p) + bias), written in batched per-partition runs.
"""

import sys

sys.path.insert(0, "/opt/trn_rl_repo")

import numpy as np
import ml_dtypes

import concourse.bass as bass
import concourse.bacc as bacc
import concourse.tile as tile
from concourse import mybir
from concourse.bass import AP

F32 = mybir.dt.float32
BF16 = mybir.dt.bfloat16
I16 = mybir.dt.int16
AF = mybir.ActivationFunctionType
OP = mybir.AluOpType
AX = mybir.AxisListType
BF16NP = ml_dtypes.bfloat16

P = 128
NEG_SLOPE = 0.2
SENT_LOGIT = -3000.0


class Cfg:
    def __init__(self, n_nodes=50000, in_ch=512, hid=64, heads=4, n_cores=8,
                 tc_max=8, node_chunk=2048, win=32768, nqueues=4,
                 single_packet=False, p1_split=0):
        self.n_nodes = n_nodes
        self.in_ch = in_ch
        self.hid = hid
        self.heads = heads
        self.hc = hid * heads                      # 256
        self.rowp = self.hc + heads                # 260 p1 psum cols
        self.n_cores = n_cores
        self.ndst = n_nodes // n_cores             # 6250
        self.nblk = -(-self.ndst // P)             # 49
        self.kt = in_ch // P                       # 4
        self.tc_max = tc_max
        self.node_chunk = node_chunk
        self.nchunks1 = -(-n_nodes // node_chunk)  # 13 p1 chunks
        self.npad = self.nchunks1 * node_chunk     # 53248 padded nodes
        self.nt = self.npad // P + 1               # 417 rows per partition
        self.rows = P * self.nt                    # 53376 table rows
        self.win = win
        self.hi_base = self.rows - win             # 20608
        assert self.hi_base <= win, "lo+hi windows must cover table"
        self.sent_t = self.nt - 1                  # sentinel col (t=416)
        self.nqueues = nqueues
        self.single_packet = single_packet
        self.ogrp = 8                              # out blocks per write
        self.p1_split = p1_split                   # p1 chunks before pass A
        # pass-A edges: src tile t < p1_split*32 (table rows written early)
        self.two_pass = p1_split > 0
        self.passa_node = p1_split * node_chunk


CFG = Cfg()


def _pidx(node, cfg):
    """Permuted table row index for node array."""
    return (node % P) * cfg.nt + node // P


def _wrap16(flat):
    """int16 index list -> dma_gather layout [128, n/16]."""
    n = len(flat)
    assert n % 16 == 0
    a = np.asarray(flat, dtype=np.int16).reshape(n // 16, 16).T
    return np.tile(a, (8, 1))


# ---------------------------------------------------------------- host prep

def host_prep_edges(edge_index, cfg):
    """Edge prep with a 4-way (block, half, pass) split.

    Category key per edge: blk*4 + half*2 + passB, where passA edges have
    src tile t < p1_split*32 (their table rows are written by the first
    p1_split phase-1 chunks).  Tile numbering = emission order: all pass-A
    tiles (block-major, lo then hi) first, then all pass-B tiles.
    """
    n = cfg.n_nodes
    loop = np.arange(n, dtype=np.int64)
    src_all = np.concatenate([edge_index[0].astype(np.int64), loop])
    dst_all = np.concatenate([edge_index[1].astype(np.int64), loop])

    ncat = 4 * cfg.nblk
    cores = []
    cnts = np.zeros((cfg.n_cores, ncat), dtype=np.int64)
    for m in range(cfg.n_cores):
        base = m * cfg.ndst
        dstr_all = (dst_all - base) % n
        own = dstr_all < cfg.ndst
        srcr = (src_all[own] - base) % n
        dstr = dstr_all[own]
        pidx = _pidx(srcr, cfg)
        is_lo = pidx < cfg.win
        passb = (srcr >= cfg.passa_node).astype(np.int64)
        blk = dstr >> 7
        rel = dstr & 127
        key = blk * 4 + (~is_lo) * 2 + passb
        order = np.argsort(key, kind="stable")
        cores.append((pidx[order], rel[order], key[order]))
        cnts[m] = np.bincount(key, minlength=ncat)

    # tiles per category, maxed over cores
    tcat = -(-cnts.max(axis=0) // P)          # [ncat]
    if cfg.two_pass:
        # guarantee pass-A lo has at least one tile per block (keeps every
        # block present in pass A so its accumulator is written)
        for b in range(cfg.nblk):
            if tcat[b * 4 + 0] + tcat[b * 4 + 2] == 0:
                tcat[b * 4 + 0] = 1
    else:
        for b in range(cfg.nblk):
            if tcat[b * 4 + 1] + tcat[b * 4 + 3] == 0:
                tcat[b * 4 + 1] = 1

    # tile numbering in emission order: pass A block-major, then pass B
    toff = np.zeros(ncat, dtype=np.int64)
    t = 0
    emit_cats = []
    for pb in (0, 1):
        for b in range(cfg.nblk):
            for hf in (0, 1):
                k = b * 4 + hf * 2 + pb
                toff[k] = t
                t += tcat[k]
                emit_cats.append(k)
    T = int(t)
    TA = int(sum(tcat[k] for k in range(ncat) if k % 2 == 0))

    # chunks in emission order; per (category), runs of <= tc_max
    chunks = []
    icol = 0
    qn = 0
    for k in emit_cats:
        b, hf, pb = k // 4, (k // 2) % 2, k % 2
        nt_cat = int(tcat[k])
        q0 = 0
        while q0 < nt_cat:
            qq = min(cfg.tc_max, nt_cat - q0)
            chunks.append(dict(b=b, t0=int(toff[k]) + q0, nt=qq, icol=icol,
                               half="hi" if hf else "lo", passb=pb,
                               qn=qn % cfg.nqueues))
            icol += qq * P // 16
            qn += 1
            q0 += qq
    icol_main = icol

    sent_lo = cfg.sent_t                     # pidx of sentinel (p=0)
    sent_hi = (P - 1) * cfg.nt + cfg.sent_t  # pidx of sentinel (p=127)

    per_core = []
    for m in range(cfg.n_cores):
        pidx, rel, key = cores[m]
        seg_cnt = np.bincount(key, minlength=ncat)
        seg_off = np.concatenate([[0], np.cumsum(seg_cnt)])
        pidx_pad = np.empty(T * P, dtype=np.int64)
        rel_pad = np.zeros(T * P, dtype=np.int64)
        for k in range(ncat):
            nt_cat = int(tcat[k])
            if nt_cat == 0:
                continue
            hf = (k // 2) % 2
            s0, s1 = int(seg_off[k]), int(seg_off[k + 1])
            ne = s1 - s0
            o0 = int(toff[k]) * P
            npad = nt_cat * P
            fill = sent_hi if hf else sent_lo
            pidx_pad[o0:o0 + npad] = fill
            pidx_pad[o0:o0 + ne] = pidx[s0:s1]
            rel_pad[o0:o0 + ne] = rel[s0:s1]
        midx = np.zeros((P, icol_main), dtype=np.int16)
        for ch in chunks:
            o0 = ch["t0"] * P
            flat = pidx_pad[o0:o0 + ch["nt"] * P].copy()
            if ch["half"] == "hi":
                flat -= cfg.hi_base
            assert flat.min() >= 0 and flat.max() < cfg.win
            midx[:, ch["icol"]:ch["icol"] + ch["nt"] * P // 16] = \
                _wrap16(flat)
        rel_t = rel_pad.reshape(T, P)                     # [tile, lane]
        rel2 = np.repeat(rel_t.T.astype(BF16NP), 2, axis=1)   # [P, 2T]
        relfm = rel_t.reshape(1, T * P).astype(BF16NP)        # [1, T*128]
        per_core.append(dict(midx=np.ascontiguousarray(midx),
                             rel2=np.ascontiguousarray(rel2),
                             relfm=np.ascontiguousarray(relfm)))
    meta = dict(chunks=chunks, T=T, TA=TA, icol_main=icol_main)
    return per_core, meta


def host_prep_weights(x, lin_w, att_src, att_dst, bias, prelu_w, cfg):
    n, ic, h, c = cfg.n_nodes, cfg.in_ch, cfg.heads, cfg.hid
    w3 = lin_w.astype(np.float64).reshape(h, c, ic)
    wa_dst = (w3 * att_dst.astype(np.float64).reshape(h, c, 1)).sum(1).T
    lwT = lin_w.astype(np.float64).T                           # [ic, 256]
    lwTi = lwT.reshape(ic, h, c).transpose(0, 2, 1).reshape(ic, h * c)
    rhs = np.concatenate([lwTi, wa_dst], axis=1)               # [ic, 260]
    rhs_w = np.ascontiguousarray(rhs.astype(BF16NP))

    att_i = att_src.astype(np.float64).reshape(h, c).T.reshape(1, h * c)
    att_bf = att_i.astype(BF16NP)
    att_rep = np.ascontiguousarray(np.broadcast_to(att_bf, (P, h * c)))
    # sentinel h row: sum_c v*att = SENT_LOGIT for every head
    att_f = att_bf.astype(np.float64).reshape(c, h)
    norm2 = (att_f * att_f).sum(axis=0)                        # [h]
    v = SENT_LOGIT * att_f / norm2[None, :]                    # [c, h]
    sent = np.ascontiguousarray(
        np.broadcast_to(v.reshape(1, h * c).astype(BF16NP), (P, h * c)))
    chk = (sent[0].astype(np.float64).reshape(c, h)
           * att_rep[0].astype(np.float64).reshape(c, h)).sum(0)
    assert chk.max() < SENT_LOGIT * 0.9, chk

    def inter(vv):
        return vv.reshape(h, c).T.reshape(h * c)
    bias_rep = np.ascontiguousarray(np.broadcast_to(
        inter(bias.astype(np.float32)), (P, h * c)))
    pw_rep = np.ascontiguousarray(np.broadcast_to(
        inter(prelu_w.astype(np.float32)), (P, h * c)))

    xbf = np.asarray(x).astype(BF16NP)
    xprep = []
    for m in range(cfg.n_cores):
        xr = np.roll(xbf, -m * cfg.ndst, axis=0)
        xpad = np.zeros((cfg.npad, ic), dtype=BF16NP)
        xpad[:n] = xr
        arr = (xpad.reshape(cfg.nchunks1, cfg.node_chunk, cfg.kt, P)
               .transpose(3, 0, 2, 1).reshape(P, -1))
        xprep.append(np.ascontiguousarray(arr))
    return dict(rhs_w=rhs_w, att_rep=att_rep, sent=sent, bias_rep=bias_rep,
                pw_rep=pw_rep, xprep=xprep)


# ---------------------------------------------------------------- builder

def build(cfg, meta, bias_nonzero=True, parts=None):
    parts = parts or {"p1", "p1mm", "p1w", "gather", "relrep", "asrc", "s1",
                      "pcomp", "mm", "epi"}
    hc, h, hid = cfg.hc, cfg.heads, cfg.hid
    T = meta["T"]
    nc = bacc.Bacc(num_swdge_queues=cfg.nqueues,
                   dynamic_dma_scratch_size=32768)

    xprep = nc.declare_dram_parameter(
        "xprep", [P, cfg.nchunks1 * cfg.kt * cfg.node_chunk], BF16,
        isOutput=False)
    rhs_w = nc.declare_dram_parameter("rhs_w", [cfg.in_ch, cfg.rowp], BF16,
                                      isOutput=False)
    att_rep = nc.declare_dram_parameter("att_rep", [P, hc], BF16,
                                        isOutput=False)
    sent = nc.declare_dram_parameter("sent", [P, hc], BF16, isOutput=False)
    bias_rep = nc.declare_dram_parameter("bias_rep", [P, hc], F32,
                                         isOutput=False)
    pw_rep = nc.declare_dram_parameter("pw_rep", [P, hc], F32, isOutput=False)
    midx = nc.declare_dram_parameter("midx", [P, meta["icol_main"]], I16,
                                     isOutput=False)
    rel2 = nc.declare_dram_parameter("rel2", [P, 2 * T], BF16, isOutput=False)
    relfm = nc.declare_dram_parameter("relfm", [1, T * P], BF16,
                                      isOutput=False)
    out = nc.declare_dram_parameter("out", [P, cfg.nblk * hc], F32,
                                    isOutput=True)

    table = nc.dram_tensor("table", [cfg.rows, hc], BF16)
    tview = table[:, :].rearrange("(p t) c -> p t c", p=P)

    with tile.TileContext(nc) as tc:
        fpool_cm = tc.tile_pool(name="fix", bufs=1)
        fpool = fpool_cm.__enter__()

        midx_sb = fpool.tile([P, meta["icol_main"]], I16)
        nc.sync.dma_start(out=midx_sb[:], in_=midx[:, :])
        rel_sb = fpool.tile([P, 2 * T], BF16)
        nc.sync.dma_start(out=rel_sb[:], in_=rel2[:, :])
        att_sb = fpool.tile([P, hc], BF16)
        nc.sync.dma_start(out=att_sb[:], in_=att_rep[:, :])
        sent_sb = fpool.tile([P, hc], BF16)
        nc.sync.dma_start(out=sent_sb[:], in_=sent[:, :])
        bias_sb = fpool.tile([P, hc], F32)
        nc.sync.dma_start(out=bias_sb[:], in_=bias_rep[:, :])
        pw_sb = fpool.tile([P, hc], F32)
        nc.sync.dma_start(out=pw_sb[:], in_=pw_rep[:, :])

        iota_i = fpool.tile([P, P], mybir.dt.int32)
        nc.gpsimd.iota(iota_i[:], pattern=[[0, P]], base=0,
                       channel_multiplier=1)
        iota_f = fpool.tile([P, P], F32)
        nc.vector.tensor_copy(out=iota_f[:], in_=iota_i[:])
        iota_p = fpool.tile([P, P], BF16)       # value = partition index
        nc.vector.tensor_copy(out=iota_p[:], in_=iota_f[:])
        iotaf_i = fpool.tile([P, P], mybir.dt.int32)
        nc.gpsimd.iota(iotaf_i[:], pattern=[[1, P]], base=0,
                       channel_multiplier=0)
        iotaf_f = fpool.tile([P, P], F32)
        nc.vector.tensor_copy(out=iotaf_f[:], in_=iotaf_i[:])
        iota_bf = fpool.tile([P, P], BF16)      # value = free index
        nc.vector.tensor_copy(out=iota_bf[:], in_=iotaf_f[:])
        ones_sb = fpool.tile([1, P], BF16)
        nc.vector.memset(ones_sb[:], 1.0)
        adst_sb = fpool.tile([P, cfg.nblk, h], BF16)

        acc_sb = (fpool.tile([P, cfg.nblk, cfg.rowp], BF16)
                  if cfg.two_pass else None)          # pass-A partials

        # chunk bookkeeping
        chunks_a = [ch for ch in meta["chunks"] if ch["passb"] == 0]
        chunks_b = [ch for ch in meta["chunks"] if ch["passb"] == 1]
        blk_a, blk_b = {}, {}
        for ch in chunks_a:
            blk_a.setdefault(ch["b"], []).append(ch)
        for ch in chunks_b:
            blk_b.setdefault(ch["b"], []).append(ch)
        # rf prefetch groups: 2 chunks per load (contiguous tile ranges)
        allch = meta["chunks"]
        rf_groups = []
        for gi in range(0, len(allch), 2):
            grp = allch[gi:gi + 2]
            gt0 = grp[0]["t0"]
            gt1 = grp[-1]["t0"] + grp[-1]["nt"]
            rf_groups.append((gt0, gt1))
            for ch in grp:
                ch["rfg"] = len(rf_groups) - 1
                ch["rfo"] = ch["t0"] - gt0
        rf_state = {}

        def do_chunk(ch, ps, start, stop, gpool, s1pool, epool, rfpool,
                     efpool, rrpool):
            qq = ch["nt"]
            t0 = ch["t0"]
            g = gpool.tile([P, qq, hc], BF16, tag="g")
            msgb = gpool.tile([P, qq, cfg.rowp], BF16, tag="msgb")
            if "gather" in parts:
                if ch["half"] == "lo":
                    in_ap = table[0:cfg.win, 0:hc]
                else:
                    in_ap = table[cfg.hi_base:cfg.rows, 0:hc]
                nc.gpsimd.dma_gather(
                    out_ap=g[:],
                    in_ap=in_ap,
                    idxs_ap=midx_sb[:, ch["icol"]:ch["icol"] + qq * P // 16],
                    num_idxs=qq * P,
                    num_idxs_reg=qq * P,
                    elem_size=hc,
                    elem_step=hc,
                    single_packet=cfg.single_packet,
                    queue_num=ch["qn"])

            # rel replicated across partitions via K=1 matmul
            s1t = s1pool.tile([P, qq, P], BF16, tag="s1t")
            if "relrep" in parts:
                gi = ch["rfg"]
                if gi not in rf_state:
                    gt0, gt1 = rf_groups[gi]
                    rft = rfpool.tile([1, (gt1 - gt0) * P], BF16, tag="rf")
                    nc.scalar.dma_start(out=rft[:],
                                        in_=relfm[0:1, gt0 * P:gt1 * P])
                    rf_state.clear()
                    rf_state[gi] = rft
                rft = rf_state[gi]
                ro = ch["rfo"] * P
                for h2 in range(0, qq, 4):
                    hn = min(4, qq - h2)
                    rr = rrpool.tile([P, 4 * P], F32, tag="rr")
                    nc.tensor.matmul(
                        rr[:, 0:hn * P],
                        lhsT=ones_sb[:, :],
                        rhs=rft[0:1, ro + h2 * P:ro + (h2 + hn) * P],
                        start=True, stop=True)
                    # PSUM f32 -> SBUF bf16 on ACT so the DVE is_equal
                    # runs in packed 2x mode
                    rrb = epool.tile([P, 4 * P], BF16, tag="rrb")
                    nc.scalar.copy(out=rrb[:, 0:hn * P], in_=rr[:, 0:hn * P])
                    iv = iota_p[:]
                    nc.vector.tensor_tensor(
                        out=s1t[:, h2:h2 + hn, :],
                        in0=AP(iv.tensor, iv.offset,
                               [iv.ap[0], [0, hn], [1, P]]),
                        in1=rrb[:, 0:hn * P].rearrange(
                            "p (q s) -> p q s", q=hn),
                        op=OP.is_equal)

            # one-hot S1[k, q, slot] = (rel[k, q] == slot)
            s1 = s1pool.tile([P, qq, P], BF16, tag="s1")
            if "s1" in parts:
                rsl = rel_sb[:, 2 * t0:2 * (t0 + qq)]
                rel_b = AP(rsl.tensor, rsl.offset,
                           [rsl.ap[0], [2, qq], [0, P // 2], [1, 2]])
                iap = iota_bf[:]
                iota_b = AP(iap.tensor, iap.offset,
                            [iap.ap[0], [0, qq], [2, P // 2], [1, 2]])
                s1v = s1[:]
                s1_b = AP(s1v.tensor, s1v.offset,
                          [s1v.ap[0], [P, qq], [2, P // 2], [1, 2]])
                nc.vector.tensor_tensor(
                    out=s1_b, in0=rel_b, in1=iota_b, op=OP.is_equal)

            # a_src = sum_c h*att: multiply, two in-place 2x folds, reduce
            asrc = epool.tile([P, qq, h], F32, tag="asrc")
            if "asrc" in parts:
                tmp = epool.tile([P, qq, hc], BF16, tag="tmp")
                av = att_sb[:]
                nc.vector.tensor_tensor(
                    out=tmp[:],
                    in0=g[:, :, 0:hc],
                    in1=AP(av.tensor, av.offset,
                           [av.ap[0], [0, qq], [1, hc]]),
                    op=OP.mult)
                f1 = epool.tile([P, qq, hc // 2], BF16, tag="f1")
                nc.vector.tensor_add(out=f1[:],
                                     in0=tmp[:, :, 0:hc // 2],
                                     in1=tmp[:, :, hc // 2:hc])
                nc.vector.tensor_add(out=f1[:, :, 0:hc // 4],
                                     in0=f1[:, :, 0:hc // 4],
                                     in1=f1[:, :, hc // 4:hc // 2])
                fv = f1[:]
                nc.vector.tensor_reduce(
                    out=asrc[:],
                    in_=AP(fv.tensor, fv.offset,
                           [fv.ap[0], [hc // 2, qq], [1, h],
                            [h, hid // 4]]),
                    axis=AX.X, op=OP.add)

            # a_dst per edge: ef_ps[:, j, :] = S1T_j^T @ adst_blk
            efps = efpool.tile([P, qq, h], F32, tag="efps")
            if "relrep" in parts:
                for j in range(qq):
                    nc.tensor.matmul(
                        efps[:, j, :],
                        lhsT=s1t[:, j, :],
                        rhs=adst_sb[:, ch["b"], :],
                        start=True, stop=True)

            if "pcomp" in parts:
                ef = epool.tile([P, qq, h], F32, tag="ef")
                nc.vector.tensor_add(out=ef[:], in0=asrc[:], in1=efps[:])
                # lrelu_0.2(x) = 0.2*x + relu(0.8*x)
                rl = epool.tile([P, qq, h], F32, tag="rl")
                nc.scalar.activation(out=rl[:], in_=ef[:], func=AF.Relu,
                                     scale=1.0 - NEG_SLOPE)
                nc.vector.scalar_tensor_tensor(
                    out=ef[:], in0=ef[:], scalar=NEG_SLOPE,
                    op0=OP.mult, in1=rl[:], op1=OP.add)
                nc.scalar.activation(out=msgb[:, :, hc:hc + h],
                                     in_=ef[:], func=AF.Exp)
                # msg = h * p (interleaved (c,hh): bcast p, stride-1 pairs)
                msg = msgb[:, :, 0:hc].rearrange("p q (c hh) -> p q c hh",
                                                 hh=h)
                gv = g[:].rearrange("p q (c hh) -> p q c hh", hh=h)
                pslice = msgb[:, :, hc:hc + h]
                pb = AP(pslice.tensor, pslice.offset,
                        [pslice.ap[0], pslice.ap[1], [0, hid], [1, h]])
                nc.vector.tensor_tensor(out=msg, in0=gv, in1=pb, op=OP.mult)

            if "mm" in parts:
                for j in range(qq):
                    nc.tensor.matmul(
                        ps[:, :],
                        lhsT=s1[:, j, :],
                        rhs=msgb[:, j, :],
                        start=(start and j == 0),
                        stop=(stop and j == qq - 1))

        def do_block_a(b, pools):
            gpool, s1pool, epool, rfpool, accpool, efpool, rrpool = pools
            chs = blk_a.get(b, [])
            if not chs:
                return
            ps = accpool.tile([P, cfg.rowp], F32, tag="psb")
            for ci, ch in enumerate(chs):
                do_chunk(ch, ps, ci == 0, ci == len(chs) - 1,
                         gpool, s1pool, epool, rfpool, efpool, rrpool)
            if "mm" in parts:
                nc.scalar.copy(out=acc_sb[:, b, :], in_=ps[:, :])

        tpc = cfg.node_chunk // P           # tiles per chunk
        ckq = cfg.kt * cfg.node_chunk       # xprep cols per chunk

        def p1_chunk(c, xpool, hpool, pspool, w_sb):
            xg = xpool.tile([P, cfg.kt, cfg.node_chunk], BF16, tag="xg")
            nc.sync.dma_start(
                out=xg[:],
                in_=xprep[:, c * ckq:(c + 1) * ckq].rearrange(
                    "p (k q) -> p k q", k=cfg.kt))
            hbuf = hpool.tile([P, tpc, hc], BF16, tag="hb")
            for t2 in (range(tpc // 2) if "p1mm" in parts else []):
                # one PSUM bank (512 f32) per node-tile slice
                ps2 = pspool.tile([P, 2, 512], F32, tag="ps")
                for i in range(2):
                    tl_ = t2 * 2 + i
                    for k in range(cfg.kt):
                        nc.tensor.matmul(
                            ps2[:, i, 0:cfg.rowp],
                            lhsT=xg[:, k, tl_ * P:(tl_ + 1) * P],
                            rhs=w_sb[:, k, :],
                            start=(k == 0), stop=(k == cfg.kt - 1))
                    tg = c * tpc + tl_
                    if tg < cfg.nblk:
                        nc.scalar.copy(out=adst_sb[:, tg, :],
                                       in_=ps2[:, i, hc:hc + h])
                # batched PSUM -> SBUF drain, alternating engines
                if t2 % 2 == 0:
                    nc.scalar.copy(out=hbuf[:, t2 * 2:(t2 + 1) * 2, :],
                                   in_=ps2[:, :, 0:hc])
                else:
                    nc.vector.tensor_copy(
                        out=hbuf[:, t2 * 2:(t2 + 1) * 2, :],
                        in_=ps2[:, :, 0:hc])
            if "p1w" in parts:
                nc.sync.dma_start(
                    out=tview[:, c * tpc:(c + 1) * tpc, :],
                    in_=hbuf[:])

        def emit_epilogue(b, ps, obpool, epool, obuf):
            src = ps
            if cfg.two_pass:
                num = obpool.tile([P, cfg.rowp], F32, tag="num")
                nc.vector.tensor_add(out=num[:], in0=ps[:, :],
                                     in1=acc_sb[:, b, :])
                src = num
            den = epool.tile([P, h], F32, tag="den")
            nc.vector.tensor_scalar_add(out=den[:], in0=src[:, hc:hc + h],
                                        scalar1=1e-6)
            rec = epool.tile([P, h], F32, tag="rec")
            nc.vector.reciprocal(out=rec[:], in_=den[:])
            ob = obpool.tile([P, hc], F32, tag="obx")
            recb = AP(rec.tensor, rec[:].offset,
                      [rec[:].ap[0], [0, hid], [1, h]])
            nc.vector.tensor_tensor(
                out=ob[:].rearrange("p (c hh) -> p c hh", hh=h),
                in0=src[:, 0:hc].rearrange("p (c hh) -> p c hh", hh=h),
                in1=recb, op=OP.mult)
            if bias_nonzero:
                nc.vector.tensor_add(out=ob[:], in0=ob[:], in1=bias_sb[:])
            t2_ = obpool.tile([P, hc], F32, tag="t2")
            nc.vector.scalar_tensor_tensor(
                out=t2_[:], in0=ob[:], scalar=0.0, op0=OP.min,
                in1=pw_sb[:], op1=OP.mult)
            # write through a permuted view: col c*4+hh -> hh*64+c
            obv = obuf[:, b % cfg.ogrp, :]
            obp_perm = AP(obv.tensor, obv.offset,
                          [obv.ap[0], [hid, h], [1, hid]])
            iview = [[1, h], [h, hid]]
            ob_i = AP(ob[:].tensor, ob[:].offset, [ob[:].ap[0]] + iview)
            t2_i = AP(t2_[:].tensor, t2_[:].offset, [t2_[:].ap[0]] + iview)
            nc.vector.scalar_tensor_tensor(
                out=obp_perm, in0=ob_i, scalar=0.0, op0=OP.max,
                in1=t2_i, op1=OP.add)

        n1 = cfg.nchunks1 if "p1" in parts else 0
        bufs_g = 4 if cfg.two_pass else 6
        bufs_se = 2 if cfg.two_pass else 3
        bufs_ef = 1 if cfg.two_pass else 3

        def emit_p1_epoch(p2pools):
            with (
                tc.tile_pool(name="p1w", bufs=1) as wpool,
                tc.tile_pool(name="p1x", bufs=2) as xpool,
                tc.tile_pool(name="p1h", bufs=2) as hpool,
                tc.tile_pool(name="p1ps", bufs=2, space="PSUM") as pspool,
            ):
                w_sb = wpool.tile([P, cfg.kt, cfg.rowp], BF16)
                nc.sync.dma_start(
                    out=w_sb[:],
                    in_=rhs_w[:, :].rearrange("(k p) r -> p k r", p=P))
                nc.sync.dma_start(
                    out=tview[:, cfg.sent_t:cfg.sent_t + 1, :],
                    in_=sent_sb[:])
                nsplit = min(cfg.p1_split, n1) if cfg.two_pass else n1
                for c in range(nsplit):
                    p1_chunk(c, xpool, hpool, pspool, w_sb)
                if cfg.two_pass:
                    tc.strict_bb_all_engine_barrier()
                    # interleave remaining p1 chunks with pass-A blocks
                    rem = list(range(nsplit, n1))
                    ablocks = list(range(cfg.nblk))
                    per = (-(-len(ablocks) // max(1, len(rem)))
                           if rem else 0)
                    ai = 0
                    for c in rem:
                        p1_chunk(c, xpool, hpool, pspool, w_sb)
                        for b in ablocks[ai:ai + per]:
                            do_block_a(b, p2pools)
                        ai += per
                    for b in ablocks[ai:]:
                        do_block_a(b, p2pools)

        if not cfg.two_pass:
            emit_p1_epoch(None)
            tc.strict_bb_all_engine_barrier()

        with (
            tc.tile_pool(name="p2g", bufs=bufs_g) as gpool,
            tc.tile_pool(name="p2s", bufs=bufs_se) as s1pool,
            tc.tile_pool(name="p2e", bufs=bufs_se) as epool,
            tc.tile_pool(name="p2r", bufs=2) as rfpool,
            tc.tile_pool(name="p2acc", bufs=2, space="PSUM") as accpool,
            tc.tile_pool(name="p2ef", bufs=bufs_ef, space="PSUM") as efpool,
            tc.tile_pool(name="p2rr", bufs=1, space="PSUM") as rrpool,
        ):
            p2pools = (gpool, s1pool, epool, rfpool, accpool, efpool,
                       rrpool)
            if cfg.two_pass:
                emit_p1_epoch(p2pools)
                tc.strict_bb_all_engine_barrier()

            # ---------------- pass B + epilogue ----------------
            with tc.tile_pool(name="p2o", bufs=2) as obpool:
                obuf = None
                for b in range(cfg.nblk):
                    chs = blk_b.get(b, [])
                    ps = accpool.tile([P, cfg.rowp], F32, tag="psb")
                    if not chs and "mm" in parts:
                        nc.vector.memset(ps[:], 0.0)
                    for ci, ch in enumerate(chs):
                        do_chunk(ch, ps, ci == 0, ci == len(chs) - 1,
                                 gpool, s1pool, epool, rfpool, efpool,
                                 rrpool)
                    if "epi" not in parts or "mm" not in parts:
                        continue
                    if b % cfg.ogrp == 0:
                        gn = min(cfg.ogrp, cfg.nblk - b)
                        obuf = obpool.tile([P, gn, hc], F32, tag="ob")
                    emit_epilogue(b, ps, obpool, epool, obuf)
                    if b % cfg.ogrp == cfg.ogrp - 1 or b == cfg.nblk - 1:
                        g0 = (b // cfg.ogrp) * cfg.ogrp
                        gn = b - g0 + 1
                        nc.sync.dma_start(
                            out=out[:, g0 * hc:(g0 + gn) * hc],
                            in_=obuf[:, 0:gn, :])
        fpool_cm.__exit__(None, None, None)
    return nc


# ---------------------------------------------------------------- runner

def _run_pjrt(nc, in_maps, n_cores, time_iters=0):
    """Compile via bass2jax and run on the 8 axon cores; optional warm
    timing via the back-to-back dispatch slope."""
    import time
    import jax
    from jax.sharding import Mesh, PartitionSpec
    from jax.experimental.shard_map import shard_map
    from concourse import bass2jax, mybir as mb

    bass2jax.install_neuronx_cc_hook()
    assert nc.dbg_addr is None
    partition_name = (nc.partition_id_tensor.name
                      if nc.partition_id_tensor else None)
    in_names, out_names, out_avals, zero_outs = [], [], [], []
    for alloc in nc.m.functions[0].allocations:
        if not isinstance(alloc, mb.MemoryLocationSet):
            continue
        name = alloc.memorylocations[0].name
        if alloc.kind == "ExternalInput":
            if name != partition_name:
                in_names.append(name)
        elif alloc.kind == "ExternalOutput":
            shape = tuple(alloc.tensor_shape)
            dtype = mb.dt.np(alloc.dtype)
            out_names.append(name)
            out_avals.append(jax.core.ShapedArray(shape, dtype))
            zero_outs.append(np.zeros(shape, dtype))
    n_params = len(in_names)
    in_names.extend(out_names)
    if partition_name is not None:
        in_names.append(partition_name)

    def _body(*args):
        operands = list(args)
        if partition_name is not None:
            operands.append(bass2jax.partition_id_tensor())
        outs = bass2jax._bass_exec_p.bind(
            *operands,
            out_avals=tuple(out_avals),
            in_names=tuple(in_names),
            out_names=tuple(out_names),
            lowering_input_output_aliases=(),
            sim_require_finite=True,
            sim_require_nnan=True,
            nc=nc,
        )
        return tuple(outs)

    devices = jax.devices()[:n_cores]
    mesh = Mesh(np.asarray(devices), ("core",))
    n_outs = len(out_avals)
    in_specs = (PartitionSpec("core"),) * (n_params + n_outs)
    out_specs = (PartitionSpec("core"),) * n_outs
    sharded = jax.jit(
        shard_map(_body, mesh=mesh, in_specs=in_specs, out_specs=out_specs,
                  check_rep=False),
        keep_unused=True,
    )
    per_core = [[np.asarray(m[name]) for name in in_names[:n_params]]
                for m in in_maps]
    concat_in = [
        np.concatenate([per_core[c][i] for c in range(n_cores)], axis=0)
        for i in range(n_params)
    ]
    concat_zeros = [
        np.zeros((n_cores * z.shape[0], *z.shape[1:]), z.dtype)
        for z in zero_outs
    ]
    sh = jax.sharding.NamedSharding(mesh, PartitionSpec("core"))
    dev_args = [jax.device_put(a, sh) for a in concat_in + concat_zeros]
    out_arrs = sharded(*dev_args)
    jax.block_until_ready(out_arrs)
    t_ns = None
    if time_iters > 0:
        def loop_wall(mreps):
            best = None
            for _ in range(time_iters):
                t0 = time.perf_counter_ns()
                o = None
                for _ in range(mreps):
                    o = sharded(*dev_args)
                jax.block_until_ready(o)
                dt = time.perf_counter_ns() - t0
                best = dt if best is None else min(best, dt)
            return best

        m1, m2 = 4, 20
        w1 = loop_wall(m1)
        w2 = loop_wall(m2)
        t_ns = max(0, (w2 - w1) // (m2 - m1))
    results = [
        {name: np.asarray(out_arrs[i]).reshape(n_cores, *out_avals[i].shape)[c]
         for i, name in enumerate(out_names)}
        for c in range(n_cores)
    ]
    return results, t_ns


def _prepare(x, edge_index, lin_w, att_src, att_dst, bias, prelu_w, cfg):
    per_core, meta = host_prep_edges(np.asarray(edge_index), cfg)
    shared = host_prep_weights(np.asarray(x), np.asarray(lin_w),
                               np.asarray(att_src), np.asarray(att_dst),
                               np.asarray(bias), np.asarray(prelu_w), cfg)
    bias_nonzero = bool(np.any(np.asarray(bias) != 0))
    nc = build(cfg, meta, bias_nonzero=bias_nonzero)
    in_maps = []
    for m in range(cfg.n_cores):
        im = dict(rhs_w=shared["rhs_w"], att_rep=shared["att_rep"],
                  sent=shared["sent"], bias_rep=shared["bias_rep"],
                  pw_rep=shared["pw_rep"], xprep=shared["xprep"][m],
                  midx=per_core[m]["midx"], rel2=per_core[m]["rel2"],
                  relfm=per_core[m]["relfm"])
        in_maps.append(im)
    return nc, in_maps


def run(x, edge_index, lin_w, att_src, att_dst, bias, prelu_w,
        cfg=None, time_iters=0):
    cfg = cfg or CFG
    nc, in_maps = _prepare(x, edge_index, lin_w, att_src, att_dst, bias,
                           prelu_w, cfg)
    nc.finalize()
    results, t_ns = _run_pjrt(nc, in_maps, cfg.n_cores,
                              time_iters=time_iters)
    outs = []
    for m in range(cfg.n_cores):
        om = results[m]["out"].reshape(P, cfg.nblk, cfg.hc)
        om = om.transpose(1, 0, 2).reshape(cfg.nblk * P, cfg.hc)
        outs.append(om[:cfg.ndst])
    full = np.concatenate(outs, axis=0).astype(np.float32)
    return full, t_ns


def kernel(**inputs):
    full, _ = run(inputs["x"], inputs["edge_index"], inputs["lin_w"],
                  inputs["att_src"], inputs["att_dst"], inputs["bias"],
                  inputs["prelu_w"])
    return full
